# revision 41
# baseline (speedup 1.0000x reference)
"""Multi-head attention (nn_MHA_76519137346007) on 8 TRN2 NeuronCores.

Reference computation (B=2, N=2048, E=1024, H=16 heads, D=64):
    Q = x @ Wq.T + bq ; K = x @ Wk.T + bk ; V = x @ Wv.T + bv
    A = softmax(Q K^T / sqrt(E))   (mask is all ones -> no-op)
    out = (A V) @ Wo.T + bo

Sharding: core c in 0..7 handles batch b = c//4 and 4 of the 16 heads
(tensor-parallel column shard of Wq/Wk/Wv, row shard of Wo). Each core
produces a partial [2048, 1024] output-projection contribution; the host
sums the 4 partials per batch and adds the constant row bv @ Wo.T + bo.

Precision: bf16 PE operands everywhere (fp32 PSUM accumulation), exp
output bf16.  Scores are ~N(0, 0.1) so exp without max-subtraction is
numerically safe.

Dataflow per core:
  qT/kT   = W^T x^T          (PE bf16, chans on partitions; all 16
                              projection groups contiguous up front —
                              interleaving them into the pipeline
                              corrupts columns on HW)
  v[t,c]  = x W^T            (PE bf16, tokens on partitions, resident x)
  sT[k,q] = kT^T qT          (PE bf16, head-paired in 64-row groups)
  pT      = exp(sT/32)       (ACT, PSUM->SBUF bf16, fused scale)
  oT_ext  = v_pad^T pT       (PE bf16; ones column drops the softmax
                              denominator in a spare PSUM row)
  epilogue (baseline-proven op shapes only): full-tile PSUM->SBUF copy,
  1-row SBUF sigma staging into a zero-padded tile, full-contraction
  ones matmul for the broadcast, full-tile copy + reciprocal + multiply
  on SBUF operands.
  y       = oT^T woT         (PE bf16 -> PSUM -> DVE copy -> DMA)

Scheduling: 32 attention quarters; S^T+exp eager, A@V lagging behind
(pT pool bounded), V-projection and Wo pieces injected from a due-date
background queue so the Tensor engine never idles long enough for the
HAM clock gate to re-throttle.
"""

import sys
from collections import deque

for _p in ("/opt/trn_rl_repo", "/root/.axon_site/_ro/trn_rl_repo"):
    if _p not in sys.path:
        sys.path.append(_p)

import numpy as np
import ml_dtypes

import concourse.bass as bass
import concourse.tile as tile
from concourse import bacc, mybir
from concourse import bass_utils

BF16 = ml_dtypes.bfloat16

B, NTOK, E, H = 2, 2048, 1024, 16
D = E // H             # 64
NCORES = 8
GPB = NCORES // B      # 4 cores per batch
HPC = H // GPB         # 4 heads per core
CH = HPC * D           # 256 channels per core
EP = E // 128          # 8 e-chunks
TC = NTOK // 128       # 16 token chunks
QB = NTOK // 512       # 4 q-blocks of 512
KC = NTOK // 128       # 16 k chunks of 128
SCALE = float(E) ** -0.5  # 1/32

_BUILT = None


def _build():
    dtb = mybir.dt.bfloat16
    dtf = mybir.dt.float32
    dtr = mybir.dt.float32r

    nc = bacc.Bacc("TRN2", target_bir_lowering=False, debug=False, num_devices=NCORES)

    xT_d = nc.dram_tensor("xT", [128, EP * NTOK], dtb, kind="ExternalInput").ap()
    wqT_d = nc.dram_tensor("wqT", [128, EP * CH], dtb, kind="ExternalInput").ap()
    wkT_d = nc.dram_tensor("wkT", [128, EP * CH], dtb, kind="ExternalInput").ap()
    wvT_d = nc.dram_tensor("wvT", [128, EP * CH], dtb, kind="ExternalInput").ap()
    woT_d = nc.dram_tensor("woT", [128, (CH // 128) * E], dtb, kind="ExternalInput").ap()
    bq_d = nc.dram_tensor("bq2", [128, CH // 128], dtf, kind="ExternalInput").ap()
    bk_d = nc.dram_tensor("bk2", [128, CH // 128], dtf, kind="ExternalInput").ap()
    onesb_d = nc.dram_tensor("onesb", [128, 1024], dtb, kind="ExternalInput").ap()
    onesf_d = nc.dram_tensor("onesf", [128, 128], dtr, kind="ExternalInput").ap()
    zeros_d = nc.dram_tensor("zeros", [128, 512], dtr, kind="ExternalInput").ap()
    y_d = nc.dram_tensor("y", [NTOK, E], dtb, kind="ExternalOutput").ap()

    with tile.TileContext(nc) as tc:
        with (
            tc.tile_pool(name="wpool", bufs=1) as wpool,
            tc.tile_pool(name="qkv", bufs=1) as qkv,
            tc.tile_pool(name="pt", bufs=6) as ptp,
            tc.tile_pool(name="small", bufs=4) as small,
            tc.tile_pool(name="st", bufs=2, space="PSUM") as stp,
            tc.tile_pool(name="acc", bufs=2, space="PSUM") as accp,
        ):
            # ---- persistent SBUF tiles ----
            wq_sb = wpool.tile([128, EP, CH], dtb, tag="wq")
            wk_sb = wpool.tile([128, EP, CH], dtb, tag="wk")
            wv_sb = wpool.tile([128, EP, CH], dtb, tag="wv")
            wo_sb = wpool.tile([128, CH // 128, E], dtb, tag="wo")
            bq_sb = wpool.tile([128, CH // 128], dtf, tag="bq")
            bk_sb = wpool.tile([128, CH // 128], dtf, tag="bk")
            ones_sb = wpool.tile([128, 128], dtr, tag="onesf")
            # zero-padded sigma staging: only rows 64 (even head) / 0 (odd
            # head) are ever rewritten; the rest stay zero so a
            # full-contraction ones matmul computes column sums = sigma
            sgE = wpool.tile([128, 512], dtr, tag="sgE")
            sgO = wpool.tile([128, 512], dtr, tag="sgO")
            xT_sb = wpool.tile([128, EP, NTOK], dtb, tag="xT")

            qT_sb = qkv.tile([128, CH // 128, NTOK], dtb, tag="qT")
            kT_sb = qkv.tile([128, CH // 128, NTOK], dtb, tag="kT")
            # v padded per head to 128 cols; the ones column drops the
            # softmax denominator into a spare PSUM row:
            #   even head: [V(64) | 1 | 1*63] -> O rows 0:64, sigma row 64
            #   odd head:  [1 | 1*63 | V(64)] -> sigma row 0, O rows 64:128
            v_sb = qkv.tile([128, TC, HPC * 128], dtb, tag="v")
            oT_sb = qkv.tile([128, CH // 128, NTOK], dtb, tag="oT")
            v4 = v_sb.rearrange("p t (h c) -> p t h c", c=128)

            # ---- input DMAs (in need order) ----
            nc.sync.dma_start(out=bq_sb, in_=bq_d)
            nc.sync.dma_start(out=bk_sb, in_=bk_d)
            nc.sync.dma_start(out=wq_sb, in_=wqT_d.rearrange("p (c n) -> p c n", n=CH))
            nc.sync.dma_start(out=wk_sb, in_=wkT_d.rearrange("p (c n) -> p c n", n=CH))
            xr = xT_d.rearrange("p (c n) -> p c n", n=NTOK)
            for tb in range(QB):
                nc.sync.dma_start(
                    out=xT_sb[:, :, tb * 512 : (tb + 1) * 512],
                    in_=xr[:, :, tb * 512 : (tb + 1) * 512],
                )
            nc.sync.dma_start(out=wv_sb, in_=wvT_d.rearrange("p (c n) -> p c n", n=CH))
            nc.sync.dma_start(out=wo_sb, in_=woT_d.rearrange("p (c n) -> p c n", n=E))
            nc.sync.dma_start(out=ones_sb, in_=onesf_d)
            nc.sync.dma_start(out=sgE, in_=zeros_d)
            nc.sync.dma_start(out=sgO, in_=zeros_d)

            # v pad initialization via DMA'd ones
            for h in range(HPC):
                col = D if h % 2 == 0 else 0
                pad0 = col + 1
                nc.sync.dma_start(out=v4[:, :, h, col], in_=onesb_d[:, 0:TC])
                nc.sync.dma_start(
                    out=v4[:, :, h, pad0 : pad0 + 63],
                    in_=onesb_d[:, 0 : TC * 63].rearrange("p (t c) -> p t c", c=63),
                )

            # ---- PE warmup: open the HAM clock gate while x streams in ----
            for w in range(8):
                psw = accp.tile([128, 512], dtf, tag="acc", name=f"warm_{w}")
                nc.tensor.matmul(
                    psw,
                    lhsT=wq_sb[:, w % 2, 0:128],
                    rhs=wq_sb.rearrange("p c n -> p (c n)")[:, 0:512],
                    start=True,
                    stop=True,
                )

            # ---- task emitters ----
            def qk_group(w_sb, b_sb, dst, mi, tb):
                pst = stp.tile([128, 3 * 512], dtf, tag="st", name="qk_ps")
                ps = pst[:, 0:512]
                for ki in range(EP):
                    nc.tensor.matmul(
                        ps,
                        lhsT=w_sb[:, ki, mi * 128 : (mi + 1) * 128],
                        rhs=xT_sb[:, ki, tb * 512 : (tb + 1) * 512],
                        start=(ki == 0),
                        stop=(ki == EP - 1),
                    )
                nc.vector.tensor_scalar_add(
                    dst[:, mi, tb * 512 : (tb + 1) * 512],
                    ps,
                    b_sb[:, mi : mi + 1],
                )

            def emit_v(ti):
                pst = stp.tile([128, 3 * 512], dtf, tag="st", name="v_ps")
                ps = pst[:, 0:CH]
                for ki in range(EP):
                    nc.tensor.matmul(
                        ps,
                        lhsT=xT_sb[:, ki, ti * 128 : (ti + 1) * 128],
                        rhs=wv_sb[:, ki, :],
                        start=(ki == 0),
                        stop=(ki == EP - 1),
                    )
                psv4 = ps.rearrange("p (h c) -> p h c", c=D)
                nc.vector.tensor_copy(out=v4[:, ti, 0::2, 0:D], in_=psv4[:, 0::2, :])
                nc.vector.tensor_copy(out=v4[:, ti, 1::2, D : 2 * D], in_=psv4[:, 1::2, :])

            def emit_st_exp(u, q, pTq):
                qb, j = u
                for grp in ((0, 1, 2), (3, 4, 5), (6, 7)):
                    st = stp.tile([128, 3 * 512], dtf, tag="st")
                    for i, s in enumerate(grp):
                        slot = q * 8 + s
                        kc, par = slot // 2, slot % 2
                        hs = par * 64
                        nc.tensor.matmul(
                            st[:, i * 512 : (i + 1) * 512],
                            lhsT=kT_sb[hs : hs + 64, j, kc * 128 : (kc + 1) * 128],
                            rhs=qT_sb[hs : hs + 64, j, qb * 512 : (qb + 1) * 512],
                            start=True,
                            stop=True,
                        )
                    g0, glen = grp[0], len(grp)
                    nc.scalar.activation(
                        out=pTq[:, g0 * 512 : (g0 + glen) * 512],
                        in_=st[:, : glen * 512],
                        func=mybir.ActivationFunctionType.Exp,
                        scale=SCALE,
                    )

            psO_tiles = {}

            def emit_av(u, q, pTq):
                qb, j = u
                if q == 0:
                    psO_e = accp.tile([128, 512], dtf, tag="acc", name=f"psOe_{qb}_{j}")
                    psO_o = accp.tile([128, 512], dtf, tag="acc", name=f"psOo_{qb}_{j}")
                    psO_tiles[u] = (psO_e, psO_o)
                for par in range(2):
                    h = 2 * j + par
                    psO = psO_tiles[u][par]
                    for kk in range(4):
                        kc = q * 4 + kk
                        nc.tensor.matmul(
                            psO,
                            lhsT=v_sb[:, kc, h * 128 : (h + 1) * 128],
                            rhs=pTq[:, (kk * 2 + par) * 512 : (kk * 2 + par + 1) * 512],
                            start=(kc == 0),
                            stop=(kc == KC - 1),
                        )

            def emit_epilogue(u):
                qb, j = u
                psO_e, psO_o = psO_tiles.pop(u)
                win = slice(qb * 512, (qb + 1) * 512)
                # Only baseline-proven op shapes: full-tile PSUM->SBUF copies
                # at base partition 0, SBUF-only 1-row staging, full-K ones
                # matmul for the sigma broadcast (K=1 matmuls and DVE PSUM
                # reads at base partition 64 both misbehave on HW).
                for par in range(2):
                    hs = par * 64
                    sig_row = D if par == 0 else 0
                    psO = psO_e if par == 0 else psO_o
                    sg = sgE if par == 0 else sgO
                    oraw = small.tile([128, 512], dtr, tag="oraw")
                    nc.vector.tensor_copy(out=oraw, in_=psO)
                    nc.vector.tensor_copy(
                        out=sg[sig_row : sig_row + 1, :],
                        in_=oraw[sig_row : sig_row + 1, :],
                    )
                    psRt = stp.tile([128, 3 * 512], dtf, tag="st", name="psR")
                    psR = psRt[:, 0:512]
                    nc.tensor.matmul(
                        psR, lhsT=ones_sb, rhs=sg, start=True, stop=True
                    )
                    rs = small.tile([128, 512], dtf, tag="rs")
                    nc.vector.tensor_copy(out=rs, in_=psR)
                    rr = small.tile([128, 512], dtf, tag="rr")
                    nc.vector.reciprocal_approx_fast(out=rr, in_=rs)
                    nc.vector.tensor_mul(
                        oT_sb[hs : hs + 64, j, win],
                        oraw[hs : hs + 64, :],
                        rr[hs : hs + 64, :],
                    )

            def emit_y(ti):
                y_sb = small.tile([128, E], dtb, tag="ysb", name="y_sb")
                for ni in range(2):
                    psYt = stp.tile([128, 3 * 512], dtf, tag="st", name="y_ps")
                    psY = psYt[:, 0:512]
                    for ci in range(CH // 128):
                        nc.tensor.matmul(
                            psY,
                            lhsT=oT_sb[:, ci, ti * 128 : (ti + 1) * 128],
                            rhs=wo_sb[:, ci, ni * 512 : (ni + 1) * 512],
                            start=(ci == 0),
                            stop=(ci == CH // 128 - 1),
                        )
                    nc.vector.tensor_copy(
                        out=y_sb[:, ni * 512 : (ni + 1) * 512], in_=psY
                    )
                nc.sync.dma_start(out=y_d[ti * 128 : (ti + 1) * 128, :], in_=y_sb)

            # ---- schedule ----
            # j-major unit order: all j=0 head pairs first
            units = [(qb, j) for j in range(2) for qb in range(4)]
            quarters = [(u, q) for u in units for q in range(4)]

            # background PE work, (due_quarter, emit_fn); kept sorted by due
            bg = deque()
            bg.append((1, lambda: qk_group(wk_sb, bk_sb, kT_sb, 0, 1)))
            bg.append((2, lambda: qk_group(wk_sb, bk_sb, kT_sb, 0, 2)))
            bg.append((3, lambda: qk_group(wk_sb, bk_sb, kT_sb, 0, 3)))
            for ti in range(TC):
                bg.append((ti // 4 + 1, lambda ti=ti: emit_v(ti)))
            for tb in range(1, QB):
                bg.append((4 * tb - 1, lambda tb=tb: qk_group(wq_sb, bq_sb, qT_sb, 0, tb)))
            bg.append((14, lambda: qk_group(wq_sb, bq_sb, qT_sb, 1, 0)))
            for tb in range(QB):
                bg.append((14 + tb, lambda tb=tb: qk_group(wk_sb, bk_sb, kT_sb, 1, tb)))
            for tb in range(1, QB):
                bg.append((15 + 4 * tb, lambda tb=tb: qk_group(wq_sb, bq_sb, qT_sb, 1, tb)))
            bg = deque(sorted(bg, key=lambda it: it[0]))

            pT_tiles = {}
            av_pending = deque()

            def do_av(t):
                u, q = quarters[t]
                emit_av(u, q, pT_tiles.pop(t))
                if q == 3:
                    emit_epilogue(u)
                    qb, j = u
                    if j == 1:
                        for ti in range(qb * 4, qb * 4 + 4):
                            bg.append((0, lambda ti=ti: emit_y(ti)))

            # preamble: the first q-block of Q and K projections; the rest
            # stream in from the background queue (safe now that the
            # epilogue has no K=1/fp32r matmuls scribbling PSUM)
            qk_group(wq_sb, bq_sb, qT_sb, 0, 0)
            qk_group(wk_sb, bk_sb, kT_sb, 0, 0)

            for t in range(len(quarters)):
                # hard bound on A@V lag: pT pool has 6 buffers
                while av_pending and av_pending[0] <= t - 6:
                    do_av(av_pending.popleft())
                u, q = quarters[t]
                pTq = ptp.tile([128, 8 * 512], dtb, tag="pt")
                pT_tiles[t] = pTq
                emit_st_exp(u, q, pTq)
                av_pending.append(t)
                while bg and bg[0][0] <= t + 1:
                    bg.popleft()[1]()
                emitted = 0
                while av_pending and av_pending[0] <= t - 1 and emitted < 2:
                    if emitted == 1 and len(av_pending) <= 4:
                        break
                    do_av(av_pending.popleft())
                    emitted += 1
            while av_pending:
                do_av(av_pending.popleft())
            while bg:
                bg.popleft()[1]()

    nc.compile()
    return nc


def _get_nc():
    global _BUILT
    if _BUILT is None:
        _BUILT = _build()
    return _BUILT


def pack(a, n):
    # [(c*128+p), n] -> [p, c*n] so each partition's data is contiguous
    c = a.shape[0] // 128
    return np.ascontiguousarray(a.reshape(c, 128, n).transpose(1, 0, 2).reshape(128, c * n))


def make_in_maps(x, Wq, bq, Wk, bk, Wv, Wo):
    maps = []
    for c in range(NCORES):
        b = c // GPB
        h0 = (c % GPB) * HPC
        sl = slice(h0 * D, h0 * D + CH)
        xTf = np.ascontiguousarray(x[b].T.astype(np.float32))
        maps.append(
            {
                "xT": pack(xTf.astype(BF16), NTOK),
                "wqT": pack(np.ascontiguousarray(Wq[sl, :].T).astype(BF16), CH),
                "wkT": pack(np.ascontiguousarray(Wk[sl, :].T).astype(BF16), CH),
                "wvT": pack(np.ascontiguousarray(Wv[sl, :].T).astype(BF16), CH),
                "woT": pack(np.ascontiguousarray(Wo[:, sl].T).astype(BF16), E),
                "bq2": np.ascontiguousarray(
                    bq[sl].astype(np.float32).reshape(CH // 128, 128).T
                ),
                "bk2": np.ascontiguousarray(
                    bk[sl].astype(np.float32).reshape(CH // 128, 128).T
                ),
                "onesb": np.ones((128, 1024), BF16),
                "onesf": np.ones((128, 128), np.float32),
                "zeros": np.zeros((128, 512), np.float32),
            }
        )
    return maps


def combine(ys, Wv_bias, Wo, bo):
    """ys: list of 8 per-core partial [NTOK, E] arrays -> [B, NTOK, E]."""
    out = np.stack(
        [sum(np.asarray(ys[b * GPB + i], np.float32) for i in range(GPB)) for b in range(B)]
    )
    out += (np.asarray(Wv_bias, np.float32) @ np.asarray(Wo, np.float32).T
            + np.asarray(bo, np.float32))[None, None, :]
    return out.astype(np.float32)


def run(x, mask, Wq, bq, Wk, bk, Wv, bv, Wo, bo, trace=False):
    """Returns (out, BassKernelResults)."""
    x = np.asarray(x, np.float32)
    maps = make_in_maps(
        x,
        np.asarray(Wq, np.float32),
        np.asarray(bq, np.float32),
        np.asarray(Wk, np.float32),
        np.asarray(bk, np.float32),
        np.asarray(Wv, np.float32),
        np.asarray(Wo, np.float32),
    )
    nc = _get_nc()
    res = bass_utils.run_bass_kernel_spmd(
        nc, maps, core_ids=list(range(NCORES)), trace=trace
    )
    ys = [res.results[c]["y"] for c in range(NCORES)]
    out = combine(ys, bv, Wo, bo)
    return out, res


def kernel(x, mask, Wq, bq, Wk, bk, Wv, bv, Wo, bo):
    out, _ = run(x, mask, Wq, bq, Wk, bk, Wv, bv, Wo, bo, trace=False)
    return out


# revision 44
# speedup vs baseline: 1.0020x; 1.0020x over previous
"""Multi-head attention (nn_MHA_76519137346007) on 8 TRN2 NeuronCores.

Reference computation (B=2, N=2048, E=1024, H=16 heads, D=64):
    Q = x @ Wq.T + bq ; K = x @ Wk.T + bk ; V = x @ Wv.T + bv
    A = softmax(Q K^T / sqrt(E))   (mask is all ones -> no-op)
    out = (A V) @ Wo.T + bo

Sharding: core c in 0..7 handles batch b = c//4 and 4 of the 16 heads
(tensor-parallel column shard of Wq/Wk/Wv, row shard of Wo). Each core
produces a partial [2048, 1024] output-projection contribution; the host
sums the 4 partials per batch and adds the constant row bv @ Wo.T + bo.

Precision: bf16 PE operands everywhere (fp32 PSUM accumulation), exp
output bf16.  Scores are ~N(0, 0.1) so exp without max-subtraction is
numerically safe.

Dataflow per core:
  qT/kT   = W^T x^T          (PE bf16, chans on partitions; all 16
                              projection groups contiguous up front —
                              interleaving them into the pipeline
                              corrupts columns on HW)
  v[t,c]  = x W^T            (PE bf16, tokens on partitions, resident x)
  sT[k,q] = kT^T qT          (PE bf16, head-paired in 64-row groups)
  pT      = exp(sT/32)       (ACT, PSUM->SBUF bf16, fused scale)
  oT_ext  = v_pad^T pT       (PE bf16; ones column drops the softmax
                              denominator in a spare PSUM row)
  epilogue (baseline-proven op shapes only): full-tile PSUM->SBUF copy,
  1-row SBUF sigma staging into a zero-padded tile, full-contraction
  ones matmul for the broadcast, full-tile copy + reciprocal + multiply
  on SBUF operands.
  y       = oT^T woT         (PE bf16 -> PSUM -> DVE copy -> DMA)

Scheduling: 32 attention quarters; S^T+exp eager, A@V lagging behind
(pT pool bounded), V-projection and Wo pieces injected from a due-date
background queue so the Tensor engine never idles long enough for the
HAM clock gate to re-throttle.
"""

import sys
from collections import deque

for _p in ("/opt/trn_rl_repo", "/root/.axon_site/_ro/trn_rl_repo"):
    if _p not in sys.path:
        sys.path.append(_p)

import numpy as np
import ml_dtypes

import concourse.bass as bass
import concourse.tile as tile
from concourse import bacc, mybir
from concourse import bass_utils

BF16 = ml_dtypes.bfloat16

B, NTOK, E, H = 2, 2048, 1024, 16
D = E // H             # 64
NCORES = 8
GPB = NCORES // B      # 4 cores per batch
HPC = H // GPB         # 4 heads per core
CH = HPC * D           # 256 channels per core
EP = E // 128          # 8 e-chunks
TC = NTOK // 128       # 16 token chunks
QB = NTOK // 512       # 4 q-blocks of 512
KC = NTOK // 128       # 16 k chunks of 128
SCALE = float(E) ** -0.5  # 1/32

_BUILT = None


def _build():
    dtb = mybir.dt.bfloat16
    dtf = mybir.dt.float32
    dtr = mybir.dt.float32r

    nc = bacc.Bacc("TRN2", target_bir_lowering=False, debug=False, num_devices=NCORES)

    xT_d = nc.dram_tensor("xT", [128, EP * NTOK], dtb, kind="ExternalInput").ap()
    wqT_d = nc.dram_tensor("wqT", [128, EP * CH], dtb, kind="ExternalInput").ap()
    wkT_d = nc.dram_tensor("wkT", [128, EP * CH], dtb, kind="ExternalInput").ap()
    wvT_d = nc.dram_tensor("wvT", [128, EP * CH], dtb, kind="ExternalInput").ap()
    woT_d = nc.dram_tensor("woT", [128, (CH // 128) * E], dtb, kind="ExternalInput").ap()
    bq_d = nc.dram_tensor("bq2", [128, CH // 128], dtf, kind="ExternalInput").ap()
    bk_d = nc.dram_tensor("bk2", [128, CH // 128], dtf, kind="ExternalInput").ap()
    onesb_d = nc.dram_tensor("onesb", [128, 1024], dtb, kind="ExternalInput").ap()
    onesf_d = nc.dram_tensor("onesf", [128, 128], dtr, kind="ExternalInput").ap()
    zeros_d = nc.dram_tensor("zeros", [128, 512], dtr, kind="ExternalInput").ap()
    y_d = nc.dram_tensor("y", [NTOK, E], dtb, kind="ExternalOutput").ap()

    with tile.TileContext(nc) as tc:
        with (
            tc.tile_pool(name="wpool", bufs=1) as wpool,
            tc.tile_pool(name="qkv", bufs=1) as qkv,
            tc.tile_pool(name="pt", bufs=6) as ptp,
            tc.tile_pool(name="small", bufs=4) as small,
            tc.tile_pool(name="st", bufs=2, space="PSUM") as stp,
            tc.tile_pool(name="acc", bufs=2, space="PSUM") as accp,
        ):
            # ---- persistent SBUF tiles ----
            wq_sb = wpool.tile([128, EP, CH], dtb, tag="wq")
            wk_sb = wpool.tile([128, EP, CH], dtb, tag="wk")
            wv_sb = wpool.tile([128, EP, CH], dtb, tag="wv")
            wo_sb = wpool.tile([128, CH // 128, E], dtb, tag="wo")
            bq_sb = wpool.tile([128, CH // 128], dtf, tag="bq")
            bk_sb = wpool.tile([128, CH // 128], dtf, tag="bk")
            ones_sb = wpool.tile([128, 128], dtr, tag="onesf")
            # zero-padded sigma staging: only rows 64 (even head) / 0 (odd
            # head) are ever rewritten; the rest stay zero so a
            # full-contraction ones matmul computes column sums = sigma
            sgE = wpool.tile([128, 512], dtr, tag="sgE")
            sgO = wpool.tile([128, 512], dtr, tag="sgO")
            xT_sb = wpool.tile([128, EP, NTOK], dtb, tag="xT")

            qT_sb = qkv.tile([128, CH // 128, NTOK], dtb, tag="qT")
            kT_sb = qkv.tile([128, CH // 128, NTOK], dtb, tag="kT")
            # v padded per head to 128 cols; the ones column drops the
            # softmax denominator into a spare PSUM row:
            #   even head: [V(64) | 1 | 1*63] -> O rows 0:64, sigma row 64
            #   odd head:  [1 | 1*63 | V(64)] -> sigma row 0, O rows 64:128
            v_sb = qkv.tile([128, TC, HPC * 128], dtb, tag="v")
            oT_sb = qkv.tile([128, CH // 128, NTOK], dtb, tag="oT")
            v4 = v_sb.rearrange("p t (h c) -> p t h c", c=128)

            # ---- input DMAs (in need order) ----
            nc.sync.dma_start(out=bq_sb, in_=bq_d)
            nc.sync.dma_start(out=bk_sb, in_=bk_d)
            nc.sync.dma_start(out=wq_sb, in_=wqT_d.rearrange("p (c n) -> p c n", n=CH))
            nc.sync.dma_start(out=wk_sb, in_=wkT_d.rearrange("p (c n) -> p c n", n=CH))
            xr = xT_d.rearrange("p (c n) -> p c n", n=NTOK)
            for tb in range(QB):
                nc.sync.dma_start(
                    out=xT_sb[:, :, tb * 512 : (tb + 1) * 512],
                    in_=xr[:, :, tb * 512 : (tb + 1) * 512],
                )
            nc.sync.dma_start(out=wv_sb, in_=wvT_d.rearrange("p (c n) -> p c n", n=CH))
            nc.sync.dma_start(out=wo_sb, in_=woT_d.rearrange("p (c n) -> p c n", n=E))
            nc.sync.dma_start(out=ones_sb, in_=onesf_d)
            nc.sync.dma_start(out=sgE, in_=zeros_d)
            nc.sync.dma_start(out=sgO, in_=zeros_d)

            # v pad initialization via DMA'd ones
            for h in range(HPC):
                col = D if h % 2 == 0 else 0
                pad0 = col + 1
                nc.sync.dma_start(out=v4[:, :, h, col], in_=onesb_d[:, 0:TC])
                nc.sync.dma_start(
                    out=v4[:, :, h, pad0 : pad0 + 63],
                    in_=onesb_d[:, 0 : TC * 63].rearrange("p (t c) -> p t c", c=63),
                )

            # ---- PE warmup: open the HAM clock gate while x streams in ----
            for w in range(8):
                psw = accp.tile([128, 512], dtf, tag="acc", name=f"warm_{w}")
                nc.tensor.matmul(
                    psw,
                    lhsT=wq_sb[:, w % 2, 0:128],
                    rhs=wq_sb.rearrange("p c n -> p (c n)")[:, 0:512],
                    start=True,
                    stop=True,
                )

            # ---- task emitters ----
            def qk_group(w_sb, b_sb, dst, mi, tb):
                pst = stp.tile([128, 3 * 512], dtf, tag="st", name="qk_ps")
                ps = pst[:, 0:512]
                for ki in range(EP):
                    nc.tensor.matmul(
                        ps,
                        lhsT=w_sb[:, ki, mi * 128 : (mi + 1) * 128],
                        rhs=xT_sb[:, ki, tb * 512 : (tb + 1) * 512],
                        start=(ki == 0),
                        stop=(ki == EP - 1),
                    )
                nc.vector.tensor_scalar_add(
                    dst[:, mi, tb * 512 : (tb + 1) * 512],
                    ps,
                    b_sb[:, mi : mi + 1],
                )

            def emit_v(ti):
                pst = stp.tile([128, 3 * 512], dtf, tag="st", name="v_ps")
                ps = pst[:, 0:CH]
                for ki in range(EP):
                    nc.tensor.matmul(
                        ps,
                        lhsT=xT_sb[:, ki, ti * 128 : (ti + 1) * 128],
                        rhs=wv_sb[:, ki, :],
                        start=(ki == 0),
                        stop=(ki == EP - 1),
                    )
                psv4 = ps.rearrange("p (h c) -> p h c", c=D)
                nc.vector.tensor_copy(out=v4[:, ti, 0::2, 0:D], in_=psv4[:, 0::2, :])
                nc.vector.tensor_copy(out=v4[:, ti, 1::2, D : 2 * D], in_=psv4[:, 1::2, :])

            def emit_st_exp(u, q, pTq):
                qb, j = u
                for grp in ((0, 1, 2), (3, 4, 5), (6, 7)):
                    st = stp.tile([128, 3 * 512], dtf, tag="st")
                    for i, s in enumerate(grp):
                        slot = q * 8 + s
                        kc, par = slot // 2, slot % 2
                        hs = par * 64
                        nc.tensor.matmul(
                            st[:, i * 512 : (i + 1) * 512],
                            lhsT=kT_sb[hs : hs + 64, j, kc * 128 : (kc + 1) * 128],
                            rhs=qT_sb[hs : hs + 64, j, qb * 512 : (qb + 1) * 512],
                            start=True,
                            stop=True,
                        )
                    g0, glen = grp[0], len(grp)
                    nc.scalar.activation(
                        out=pTq[:, g0 * 512 : (g0 + glen) * 512],
                        in_=st[:, : glen * 512],
                        func=mybir.ActivationFunctionType.Exp,
                        scale=SCALE,
                    )

            psO_tiles = {}

            def emit_av(u, q, pTq):
                qb, j = u
                if q == 0:
                    psO_e = accp.tile([128, 512], dtf, tag="acc", name=f"psOe_{qb}_{j}")
                    psO_o = accp.tile([128, 512], dtf, tag="acc", name=f"psOo_{qb}_{j}")
                    psO_tiles[u] = (psO_e, psO_o)
                for par in range(2):
                    h = 2 * j + par
                    psO = psO_tiles[u][par]
                    for kk in range(4):
                        kc = q * 4 + kk
                        nc.tensor.matmul(
                            psO,
                            lhsT=v_sb[:, kc, h * 128 : (h + 1) * 128],
                            rhs=pTq[:, (kk * 2 + par) * 512 : (kk * 2 + par + 1) * 512],
                            start=(kc == 0),
                            stop=(kc == KC - 1),
                        )

            def emit_epilogue(u):
                qb, j = u
                psO_e, psO_o = psO_tiles.pop(u)
                win = slice(qb * 512, (qb + 1) * 512)
                # Only baseline-proven op shapes: full-tile PSUM->SBUF copies
                # at base partition 0, SBUF-only 1-row staging, full-K ones
                # matmul for the sigma broadcast (K=1 matmuls and DVE PSUM
                # reads at base partition 64 both misbehave on HW).
                for par in range(2):
                    hs = par * 64
                    sig_row = D if par == 0 else 0
                    psO = psO_e if par == 0 else psO_o
                    sg = sgE if par == 0 else sgO
                    oraw = small.tile([128, 512], dtr, tag="oraw")
                    nc.vector.tensor_copy(out=oraw, in_=psO)
                    nc.vector.tensor_copy(
                        out=sg[sig_row : sig_row + 1, :],
                        in_=oraw[sig_row : sig_row + 1, :],
                    )
                    psRt = stp.tile([128, 3 * 512], dtf, tag="st", name="psR")
                    psR = psRt[:, 0:512]
                    nc.tensor.matmul(
                        psR, lhsT=ones_sb, rhs=sg, start=True, stop=True
                    )
                    rs = small.tile([128, 512], dtf, tag="rs")
                    nc.vector.tensor_copy(out=rs, in_=psR)
                    rr = small.tile([128, 512], dtf, tag="rr")
                    nc.vector.reciprocal_approx_fast(out=rr, in_=rs)
                    nc.vector.tensor_mul(
                        oT_sb[hs : hs + 64, j, win],
                        oraw[hs : hs + 64, :],
                        rr[hs : hs + 64, :],
                    )

            def emit_y(ti):
                y_sb = small.tile([128, E], dtb, tag="ysb", name="y_sb")
                for ni in range(2):
                    psYt = stp.tile([128, 3 * 512], dtf, tag="st", name="y_ps")
                    psY = psYt[:, 0:512]
                    for ci in range(CH // 128):
                        nc.tensor.matmul(
                            psY,
                            lhsT=oT_sb[:, ci, ti * 128 : (ti + 1) * 128],
                            rhs=wo_sb[:, ci, ni * 512 : (ni + 1) * 512],
                            start=(ci == 0),
                            stop=(ci == CH // 128 - 1),
                        )
                    nc.vector.tensor_copy(
                        out=y_sb[:, ni * 512 : (ni + 1) * 512], in_=psY
                    )
                nc.sync.dma_start(out=y_d[ti * 128 : (ti + 1) * 128, :], in_=y_sb)

            # ---- schedule ----
            # j-major unit order: all j=0 head pairs first
            units = [(qb, j) for j in range(2) for qb in range(4)]
            quarters = [(u, q) for u in units for q in range(4)]

            # background PE work, (due_quarter, emit_fn); kept sorted by due
            bg = deque()
            for ti in range(TC):
                bg.append((ti // 4 + 1, lambda ti=ti: emit_v(ti)))
            bg = deque(sorted(bg, key=lambda it: it[0]))

            pT_tiles = {}
            av_pending = deque()

            def do_av(t):
                u, q = quarters[t]
                emit_av(u, q, pT_tiles.pop(t))
                if q == 3:
                    emit_epilogue(u)
                    qb, j = u
                    if j == 1:
                        for ti in range(qb * 4, qb * 4 + 4):
                            bg.append((0, lambda ti=ti: emit_y(ti)))

            # preamble: ALL Q/K projection groups (contiguous; interleaving
            # them into the attention pipeline corrupts qT/kT columns on HW)
            for tb in range(QB):
                qk_group(wq_sb, bq_sb, qT_sb, 0, tb)
                qk_group(wk_sb, bk_sb, kT_sb, 0, tb)
            for tb in range(QB):
                qk_group(wq_sb, bq_sb, qT_sb, 1, tb)
                qk_group(wk_sb, bk_sb, kT_sb, 1, tb)

            for t in range(len(quarters)):
                # hard bound on A@V lag: pT pool has 6 buffers
                while av_pending and av_pending[0] <= t - 6:
                    do_av(av_pending.popleft())
                u, q = quarters[t]
                pTq = ptp.tile([128, 8 * 512], dtb, tag="pt")
                pT_tiles[t] = pTq
                emit_st_exp(u, q, pTq)
                av_pending.append(t)
                while bg and bg[0][0] <= t + 1:
                    bg.popleft()[1]()
                emitted = 0
                while av_pending and av_pending[0] <= t - 1 and emitted < 2:
                    if emitted == 1 and len(av_pending) <= 4:
                        break
                    do_av(av_pending.popleft())
                    emitted += 1
            while av_pending:
                do_av(av_pending.popleft())
            while bg:
                bg.popleft()[1]()

    nc.compile()
    return nc


def _get_nc():
    global _BUILT
    if _BUILT is None:
        _BUILT = _build()
    return _BUILT


def pack(a, n):
    # [(c*128+p), n] -> [p, c*n] so each partition's data is contiguous
    c = a.shape[0] // 128
    return np.ascontiguousarray(a.reshape(c, 128, n).transpose(1, 0, 2).reshape(128, c * n))


def make_in_maps(x, Wq, bq, Wk, bk, Wv, Wo):
    maps = []
    for c in range(NCORES):
        b = c // GPB
        h0 = (c % GPB) * HPC
        sl = slice(h0 * D, h0 * D + CH)
        xTf = np.ascontiguousarray(x[b].T.astype(np.float32))
        maps.append(
            {
                "xT": pack(xTf.astype(BF16), NTOK),
                "wqT": pack(np.ascontiguousarray(Wq[sl, :].T).astype(BF16), CH),
                "wkT": pack(np.ascontiguousarray(Wk[sl, :].T).astype(BF16), CH),
                "wvT": pack(np.ascontiguousarray(Wv[sl, :].T).astype(BF16), CH),
                "woT": pack(np.ascontiguousarray(Wo[:, sl].T).astype(BF16), E),
                "bq2": np.ascontiguousarray(
                    bq[sl].astype(np.float32).reshape(CH // 128, 128).T
                ),
                "bk2": np.ascontiguousarray(
                    bk[sl].astype(np.float32).reshape(CH // 128, 128).T
                ),
                "onesb": np.ones((128, 1024), BF16),
                "onesf": np.ones((128, 128), np.float32),
                "zeros": np.zeros((128, 512), np.float32),
            }
        )
    return maps


def combine(ys, Wv_bias, Wo, bo):
    """ys: list of 8 per-core partial [NTOK, E] arrays -> [B, NTOK, E]."""
    out = np.stack(
        [sum(np.asarray(ys[b * GPB + i], np.float32) for i in range(GPB)) for b in range(B)]
    )
    out += (np.asarray(Wv_bias, np.float32) @ np.asarray(Wo, np.float32).T
            + np.asarray(bo, np.float32))[None, None, :]
    return out.astype(np.float32)


def run(x, mask, Wq, bq, Wk, bk, Wv, bv, Wo, bo, trace=False):
    """Returns (out, BassKernelResults)."""
    x = np.asarray(x, np.float32)
    maps = make_in_maps(
        x,
        np.asarray(Wq, np.float32),
        np.asarray(bq, np.float32),
        np.asarray(Wk, np.float32),
        np.asarray(bk, np.float32),
        np.asarray(Wv, np.float32),
        np.asarray(Wo, np.float32),
    )
    nc = _get_nc()
    res = bass_utils.run_bass_kernel_spmd(
        nc, maps, core_ids=list(range(NCORES)), trace=trace
    )
    ys = [res.results[c]["y"] for c in range(NCORES)]
    out = combine(ys, bv, Wo, bo)
    return out, res


def kernel(x, mask, Wq, bq, Wk, bk, Wv, bv, Wo, bo):
    out, _ = run(x, mask, Wq, bq, Wk, bk, Wv, bv, Wo, bo, trace=False)
    return out
